# revision 14
# baseline (speedup 1.0000x reference)
"""Trainium2 Bass kernel for nn_Conv2d_24833500905755 (3x3 conv, B=32,
C_in=64, C_out=128, 56x56, pad 1, with the reference's mismatched
weight-flatten order).

Math: out[b,co,h,w] = sum_{c,di,dj} xpad[b,c,h+di,w+dj] * Wt[c,di*3+dj,co]
with Wt = K.reshape(576, C_OUT).reshape(C_IN, 9, C_OUT).

Data-parallel: 4 images per NeuronCore, 2 images packed on the
128-partition dim (fp16 matmuls, K=64 contraction per half, concurrent
PE row-group tiles). Raw-bass hand-scheduled engine programs.

v5 scheduling model (calibrated from perfetto traces):
  - All engine streams are released from the framework preamble at
    ~7.2-7.9us; the first PE instruction lands ~7.3us.  DMA chain per
    instruction: issue 0.6-0.7 + hwdge/dge 1.3 + transfer (360 GB/s
    aggregate) + 0.9 sem propagation -> first input pieces land ~10.5.
  - Critical pieces (xA rows 0:10 on sync, w taps 0-3 on scalar) are
    issued pre-block from the earliest slots; w taps 4-8 / xC second
    on the same engines; xB from gpsimd (released late ~7.9).
  - PE p-state: half clock for ~6us from first matmul, resets on idle.
    ALL junk runs pre-block so the tensor's block-entry-barrier arrival
    IS the junk/real boundary (~10.9), with DMA sems pre-satisfied ->
    no PE gap, ramp completes ~13.2.
  - Tail: pair-1 ends with two 4-row minichunks; copies split across
    vector (h0) / scalar (h1), DMAs across sync (h0) / gpsimd+scalar
    (h1) to minimize the exposed serial chain after the last matmul.

Output is fp16 on-chip and in HBM; host upcasts to fp32.
"""

from contextlib import ExitStack

import numpy as np

import concourse.bass as bass
import concourse.mybir as mybir
from concourse.bass_utils import run_bass_kernel_spmd

B, C_IN, C_OUT, H = 32, 64, 128, 56
KS = 3
N_CORES = 8
BPC = B // N_CORES
HP = H + 2
RCHUNK = 8
NCHUNK = H // RCHUNK          # 7 chunks/image
OBLOCKS0 = [(0, 24), (24, 40), (40, 48), (48, 56)]
OBLOCKS1 = [(0, 24), (24, 32), (32, 40), (40, 48)]  # + minis (48,52),(52,56)
MM_DT = mybir.dt.float16
OUT_DT = mybir.dt.float16
N_JUNK_BIG = 16               # pre-block 448-col junks (8 pair-slots w/ probe)
N_JUNK_SMALL = 10             # pre-block 56-col junks (5 pair-slots, granular)


def _block_of(blocks, h0):
    for bi, (blo, bhi) in enumerate(blocks):
        if blo <= h0 < bhi:
            return bi, blo, bhi
    raise AssertionError(h0)


def build_nc(mm_dt=MM_DT):
    f32 = mybir.dt.float32
    nc = bass.Bass()
    x_ext = nc.declare_dram_parameter("x", [BPC, C_IN, HP, HP], mm_dt, isOutput=False)
    w_ext = nc.declare_dram_parameter("w", [2 * C_IN, KS * KS, C_OUT], mm_dt, isOutput=False)
    out_ext = nc.declare_dram_parameter("out", [BPC, C_OUT, H, H], OUT_DT, isOutput=True)

    # out DMAs: pair0 4 blocks x2 halves + pair1 (4 blocks + 2 minis) x2
    n_out_dmas = len(OBLOCKS0) * 2 + (len(OBLOCKS1) + 2) * 2

    with ExitStack() as ctx:
        wt = ctx.enter_context(nc.sbuf_tensor("wt", [2 * C_IN, KS * KS, C_OUT], mm_dt))
        xps = [
            ctx.enter_context(nc.sbuf_tensor(f"xp{p}", [2 * C_IN, HP, HP], mm_dt))
            for p in range(2)
        ]
        obs = [
            [
                [
                    ctx.enter_context(
                        nc.sbuf_tensor(f"ob_{p}_{h}_{bi}", [C_OUT, bhi - blo, H], OUT_DT)
                    )
                    for bi, (blo, bhi) in enumerate(
                        OBLOCKS0 if p == 0 else OBLOCKS1 + [(48, 52), (52, 56)]
                    )
                ]
                for h in range(2)
            ]
            for p in range(2)
        ]
        banks = [
            [
                ctx.enter_context(
                    nc.psum_tensor(f"ps_{s}_{h}", [C_OUT, RCHUNK, H], f32)
                )
                for h in range(2)
            ]
            for s in range(4)
        ]
        # minichunk banks alias retired full banks: mini0 -> banks[3]
        # (last used by chunk 11), mini1 -> banks[1] (chunk 9)
        minib = [
            [banks[3][h][:, 0:4, :] for h in range(2)],
            [banks[1][h][:, 0:4, :] for h in range(2)],
        ]
        s_w = ctx.enter_context(nc.semaphore("s_w"))
        s_xa = ctx.enter_context(nc.semaphore("s_xa"))
        s_xb = ctx.enter_context(nc.semaphore("s_xb"))
        s_xc = ctx.enter_context(nc.semaphore("s_xc"))
        s_x1 = ctx.enter_context(nc.semaphore("s_x1"))
        s_mm = ctx.enter_context(nc.semaphore("s_mm"))
        s_cp = ctx.enter_context(nc.semaphore("s_cp"))
        s_cp2 = ctx.enter_context(nc.semaphore("s_cp2"))
        s_out = ctx.enter_context(nc.semaphore("s_out"))

        src0 = x_ext[0:2].rearrange("b c h w -> (b c) h w")
        src1 = x_ext[2:4].rearrange("b c h w -> (b c) h w")
        dsts = [
            out_ext[i : i + 1].rearrange("b c h w -> (b c) h w") for i in range(BPC)
        ]

        # ---- pre-block: critical DMA issues on the earliest slots ----
        nc.sync.dma_start(out=xps[0][:, 0:10, :], in_=src0[:, 0:10, :]).then_inc(s_xa, 16)
        nc.scalar.dma_start(out=wt[:, 0:4, :], in_=w_ext[:, 0:4, :]).then_inc(s_w, 16)
        nc.scalar.dma_start(out=wt[:, 4:, :], in_=w_ext[:, 4:, :]).then_inc(s_w, 16)
        nc.sync.dma_start(out=xps[0][:, 34:HP, :], in_=src0[:, 34:HP, :]).then_inc(s_xc, 16)
        nc.gpsimd.dma_start(out=xps[0][:, 10:34, :], in_=src0[:, 10:34, :]).then_inc(s_xb, 16)

        # ---- pre-block: the whole junk ramp bridge; the tensor's block-
        # entry-barrier arrival (~junk end) is the junk/real boundary, with
        # all first-chunk DMA sems already satisfied -> no PE gap.
        # banks[3] junk contents are cleared by chunk 3's start=True.
        for wi in range(N_JUNK_BIG):
            h = wi % 2
            c0 = h * C_IN
            nc.tensor.matmul(
                out=banks[3][h][:],
                lhsT=wt[c0 : c0 + C_IN, 0, :],
                rhs=xps[0][c0 : c0 + C_IN, 0:RCHUNK, 0:H],
                start=True,
                stop=True,
            )
        for wi in range(N_JUNK_SMALL):
            h = wi % 2
            c0 = h * C_IN
            nc.tensor.matmul(
                out=banks[3][h][:, 0:1, :],
                lhsT=wt[c0 : c0 + C_IN, 0, :],
                rhs=xps[0][c0 : c0 + C_IN, 0:1, 0:H],
                start=True,
                stop=True,
            )

        with nc.Block() as block:

            @block.sync
            def _(sync: bass.BassEngine):
                for p in range(2):
                    blocks = OBLOCKS0 if p == 0 else OBLOCKS1 + [(48, 52), (52, 56)]
                    for bi, (blo, bhi) in enumerate(blocks):
                        if p == 0:
                            ticks = bhi // RCHUNK
                        elif bhi <= 48:
                            ticks = NCHUNK + bhi // RCHUNK
                        else:
                            ticks = 14 if bhi == 52 else 15
                        sync.wait_ge(s_cp, ticks)
                        sync.dma_start(
                            out=dsts[2 * p][:, blo:bhi, :], in_=obs[p][0][bi][:]
                        ).then_inc(s_out, 16)
                sync.wait_ge(s_out, 16 * n_out_dmas)

            @block.scalar
            def _(scalar: bass.BassEngine):
                cp2 = 0
                for p in range(2):
                    blocks = OBLOCKS0 if p == 0 else OBLOCKS1
                    for ci in range(NCHUNK):
                        c = p * NCHUNK + ci
                        h0 = ci * RCHUNK
                        if p == 1 and ci == NCHUNK - 1:
                            break  # minichunks below
                        bi, blo, bhi = _block_of(blocks, h0)
                        scalar.wait_ge(s_mm, 2 * (c + 1))
                        scalar.copy(
                            out=obs[p][1][bi][:, h0 - blo : h0 - blo + RCHUNK, :],
                            in_=banks[c % 4][1][:],
                        ).then_inc(s_cp2, 1)
                        cp2 += 1
                        if h0 + RCHUNK == bhi:
                            scalar.wait_ge(s_cp2, cp2)
                            scalar.dma_start(
                                out=dsts[2 * p + 1][:, blo:bhi, :], in_=obs[p][1][bi][:]
                            ).then_inc(s_out, 16)
                # minichunk h1 copies; mini0's DMA goes to gpsimd so the
                # scalar chain after the last matmul is copy+DMA once
                for mc in range(2):
                    bi = len(OBLOCKS1) + mc
                    scalar.wait_ge(s_mm, 28 + 2 * mc)
                    scalar.copy(out=obs[1][1][bi][:], in_=minib[mc][1]).then_inc(s_cp2, 1)
                    cp2 += 1
                scalar.wait_ge(s_cp2, cp2)
                scalar.dma_start(
                    out=dsts[3][:, 52:56, :], in_=obs[1][1][len(OBLOCKS1) + 1][:]
                ).then_inc(s_out, 16)

            @block.gpsimd
            def _(gpsimd: bass.BassEngine):
                gpsimd.wait_ge(s_mm, 2)
                gpsimd.dma_start(out=xps[1][:, 0:12, :], in_=src1[:, 0:12, :]).then_inc(s_x1, 16)
                gpsimd.dma_start(out=xps[1][:, 12:34, :], in_=src1[:, 12:34, :]).then_inc(s_x1, 16)
                gpsimd.dma_start(out=xps[1][:, 34:HP, :], in_=src1[:, 34:HP, :]).then_inc(s_x1, 16)
                # mini0 h1 output DMA (scalar handles mini1's)
                gpsimd.wait_ge(s_cp2, 14)
                gpsimd.dma_start(
                    out=dsts[3][:, 48:52, :], in_=obs[1][1][len(OBLOCKS1)][:]
                ).then_inc(s_out, 16)

            @block.tensor
            def _(tensor: bass.BassEngine):
                for p in range(2):
                    for ci in range(NCHUNK):
                        c = p * NCHUNK + ci
                        h0 = ci * RCHUNK
                        if p == 1 and ci == NCHUNK - 1:
                            break  # minichunks below
                        if p == 0:
                            if ci == 0:
                                tensor.wait_ge(s_w, 16)   # taps 0-3
                                tensor.wait_ge(s_xa, 16)  # rows [0,10)
                            elif ci == 1:
                                tensor.wait_ge(s_xb, 16)  # rows [10,34)
                            elif ci == 4:
                                tensor.wait_ge(s_xc, 16)  # rows [34,58)
                        else:
                            if ci == 0:
                                tensor.wait_ge(s_x1, 16)
                            elif ci == 1:
                                tensor.wait_ge(s_x1, 32)
                            elif ci == 4:
                                tensor.wait_ge(s_x1, 48)
                        if c >= 4:
                            # WAR: bank slot c%4 last used by chunk c-4
                            tensor.wait_ge(s_cp, c - 3)
                            tensor.wait_ge(s_cp2, c - 3)
                        for k in range(KS * KS):
                            di, dj = divmod(k, KS)
                            last = k == KS * KS - 1
                            if p == 0 and ci == 0 and k == 4:
                                tensor.wait_ge(s_w, 32)  # taps 4-8
                            for half in range(2):
                                c0 = half * C_IN
                                mm = tensor.matmul(
                                    out=banks[c % 4][half][:],
                                    lhsT=wt[c0 : c0 + C_IN, k, :],
                                    rhs=xps[p][
                                        c0 : c0 + C_IN,
                                        h0 + di : h0 + di + RCHUNK,
                                        dj : dj + H,
                                    ],
                                    start=(k == 0),
                                    stop=last,
                                )
                                if last and half == 1:
                                    mm.then_inc(s_mm, 2)
                # pair-1 rows 48-56 as two 4-row minichunks
                for mc in range(2):
                    h0 = 48 + 4 * mc
                    if mc == 0:
                        # WAR: banks[3] last written by chunk 11 (tick 12),
                        # banks[1] by chunk 9 (tick 10) — both long done
                        tensor.wait_ge(s_cp, 12)
                        tensor.wait_ge(s_cp2, 12)
                    for k in range(KS * KS):
                        di, dj = divmod(k, KS)
                        last = k == KS * KS - 1
                        for half in range(2):
                            c0 = half * C_IN
                            mm = tensor.matmul(
                                out=minib[mc][half],
                                lhsT=wt[c0 : c0 + C_IN, k, :],
                                rhs=xps[1][
                                    c0 : c0 + C_IN,
                                    h0 + di : h0 + di + 4,
                                    dj : dj + H,
                                ],
                                start=(k == 0),
                                stop=last,
                            )
                            if last and half == 1:
                                mm.then_inc(s_mm, 2)

            @block.vector
            def _(vector: bass.BassEngine):
                for p in range(2):
                    blocks = OBLOCKS0 if p == 0 else OBLOCKS1
                    for ci in range(NCHUNK):
                        c = p * NCHUNK + ci
                        h0 = ci * RCHUNK
                        if p == 1 and ci == NCHUNK - 1:
                            break
                        bi, blo, bhi = _block_of(blocks, h0)
                        vector.wait_ge(s_mm, 2 * (c + 1))
                        vector.tensor_copy(
                            out=obs[p][0][bi][:, h0 - blo : h0 - blo + RCHUNK, :],
                            in_=banks[c % 4][0][:],
                        ).then_inc(s_cp, 1)
                for mc in range(2):
                    bi = len(OBLOCKS1) + mc
                    vector.wait_ge(s_mm, 28 + 2 * mc)
                    vector.tensor_copy(out=obs[1][0][bi][:], in_=minib[mc][0]).then_inc(s_cp, 1)

    return nc


def _prep_inputs(x, K, mm_dt=MM_DT):
    np_dt = mybir.dt.np(mm_dt)
    x = np.ascontiguousarray(np.asarray(x, dtype=np.float32))
    K = np.ascontiguousarray(np.asarray(K, dtype=np.float32))
    xpad = np.pad(x, ((0, 0), (0, 0), (1, 1), (1, 1))).astype(np_dt)
    Wt = K.reshape(KS * KS * C_IN, C_OUT).reshape(C_IN, KS * KS, C_OUT)
    Wrep = np.ascontiguousarray(np.concatenate([Wt, Wt], axis=0)).astype(np_dt)
    shards = xpad.reshape(N_CORES, BPC, C_IN, HP, HP)
    return [{"x": np.ascontiguousarray(shards[i]), "w": Wrep} for i in range(N_CORES)]


def run(x, K, trace=False, mm_dt=MM_DT):
    nc = build_nc(mm_dt)
    in_maps = _prep_inputs(x, K, mm_dt)
    res = run_bass_kernel_spmd(nc, in_maps, list(range(N_CORES)), trace=trace)
    out = np.concatenate([res.results[i]["out"] for i in range(N_CORES)], axis=0)
    return out.astype(np.float32), res


def kernel(x, K):
    out, _ = run(x, K, trace=False)
    return out


# revision 19
# speedup vs baseline: 1.1169x; 1.1169x over previous
"""Trainium2 Bass kernel for nn_Conv2d_24833500905755 (3x3 conv, B=32,
C_in=64, C_out=128, 56x56, pad 1, with the reference's mismatched
weight-flatten order).

Math: out[b,co,h,w] = sum_{c,di,dj} xpad[b,c,h+di,w+dj] * Wt[c,di*3+dj,co]
with Wt = K.reshape(576, C_OUT).reshape(C_IN, 9, C_OUT).

Data-parallel: 4 images per NeuronCore, 2 images packed on the
128-partition dim (fp16 matmuls, K=64 contraction per half, concurrent
PE row-group tiles). Raw-bass hand-scheduled engine programs.

v5 scheduling model (calibrated from perfetto traces):
  - All engine streams are released from the framework preamble at
    ~7.2-7.9us; the first PE instruction lands ~7.3us.  DMA chain per
    instruction: issue 0.6-0.7 + hwdge/dge 1.3 + transfer (360 GB/s
    aggregate) + 0.9 sem propagation -> first input pieces land ~10.5.
  - Critical pieces (xA rows 0:10 on sync, w taps 0-3 on scalar) are
    issued pre-block from the earliest slots; w taps 4-8 / xC second
    on the same engines; xB from gpsimd (released late ~7.9).
  - PE p-state: half clock for ~6us from first matmul, resets on idle.
    ALL junk runs pre-block so the tensor's block-entry-barrier arrival
    IS the junk/real boundary (~10.9), with DMA sems pre-satisfied ->
    no PE gap, ramp completes ~13.2.
  - Tail: pair-1 ends with two 4-row minichunks; copies split across
    vector (h0) / scalar (h1), DMAs across sync (h0) / gpsimd+scalar
    (h1) to minimize the exposed serial chain after the last matmul.

Output is fp16 on-chip and in HBM; host upcasts to fp32.
"""

from contextlib import ExitStack

import numpy as np

import concourse.bass as bass
import concourse.mybir as mybir
from concourse.bass_utils import run_bass_kernel_spmd

B, C_IN, C_OUT, H = 32, 64, 128, 56
KS = 3
N_CORES = 8
BPC = B // N_CORES
HP = H + 2
RCHUNK = 8
NCHUNK = H // RCHUNK          # 7 chunks/image
OBLOCKS0 = [(0, 24), (24, 40), (40, 48), (48, 56)]
OBLOCKS1 = [(0, 24), (24, 32), (32, 40), (40, 48)]  # + minis (48,52),(52,56)
MM_DT = mybir.dt.float16
OUT_DT = mybir.dt.float16
N_JUNK_BIG = 18               # pre-block 448-col junks (9 pair-slots)
N_JUNK_SMALL = 8              # pre-block 56-col junks (4 pair-slots, granular)


def _block_of(blocks, h0):
    for bi, (blo, bhi) in enumerate(blocks):
        if blo <= h0 < bhi:
            return bi, blo, bhi
    raise AssertionError(h0)


def build_nc(mm_dt=MM_DT):
    f32 = mybir.dt.float32
    nc = bass.Bass()
    x_ext = nc.declare_dram_parameter("x", [BPC, C_IN, HP, HP], mm_dt, isOutput=False)
    w_ext = nc.declare_dram_parameter("w", [2 * C_IN, KS * KS, C_OUT], mm_dt, isOutput=False)
    out_ext = nc.declare_dram_parameter("out", [BPC, C_OUT, H, H], OUT_DT, isOutput=True)

    # out DMAs: pair0 4 blocks x2 halves + pair1 (4 blocks + 2 minis) x2
    n_out_dmas = len(OBLOCKS0) * 2 + (len(OBLOCKS1) + 2) * 2

    with ExitStack() as ctx:
        wt = ctx.enter_context(nc.sbuf_tensor("wt", [2 * C_IN, KS * KS, C_OUT], mm_dt))
        xps = [
            ctx.enter_context(nc.sbuf_tensor(f"xp{p}", [2 * C_IN, HP, HP], mm_dt))
            for p in range(2)
        ]
        obs = [
            [
                [
                    ctx.enter_context(
                        nc.sbuf_tensor(f"ob_{p}_{h}_{bi}", [C_OUT, bhi - blo, H], OUT_DT)
                    )
                    for bi, (blo, bhi) in enumerate(
                        OBLOCKS0 if p == 0 else OBLOCKS1 + [(48, 52), (52, 56)]
                    )
                ]
                for h in range(2)
            ]
            for p in range(2)
        ]
        banks = [
            [
                ctx.enter_context(
                    nc.psum_tensor(f"ps_{s}_{h}", [C_OUT, RCHUNK, H], f32)
                )
                for h in range(2)
            ]
            for s in range(4)
        ]
        # minichunk banks alias retired full banks: mini0 -> banks[3]
        # (last used by chunk 11), mini1 -> banks[1] (chunk 9)
        minib = [
            [banks[3][h][:, 0:4, :] for h in range(2)],
            [banks[1][h][:, 0:4, :] for h in range(2)],
        ]
        s_w1 = ctx.enter_context(nc.semaphore("s_w1"))
        s_w2 = ctx.enter_context(nc.semaphore("s_w2"))
        s_xa = ctx.enter_context(nc.semaphore("s_xa"))
        s_xb = ctx.enter_context(nc.semaphore("s_xb"))
        s_xc = ctx.enter_context(nc.semaphore("s_xc"))
        s_x1 = ctx.enter_context(nc.semaphore("s_x1"))
        s_mm = ctx.enter_context(nc.semaphore("s_mm"))
        s_cp = ctx.enter_context(nc.semaphore("s_cp"))
        s_cp2 = ctx.enter_context(nc.semaphore("s_cp2"))
        s_out = ctx.enter_context(nc.semaphore("s_out"))

        src0 = x_ext[0:2].rearrange("b c h w -> (b c) h w")
        src1 = x_ext[2:4].rearrange("b c h w -> (b c) h w")
        dsts = [
            out_ext[i : i + 1].rearrange("b c h w -> (b c) h w") for i in range(BPC)
        ]

        # ---- pre-block: critical DMA issues ----
        # HWDGE descriptor-gen (~0.63us/DMA) is a single shared device and
        # serializes across sync/scalar DMAs in issue order; gpsimd's SWDGE
        # gen runs on the Pool engine in parallel.  Order: xA gen first,
        # w2 gen second, xB/xC gens later; w1 via SWDGE concurrently.
        nc.sync.dma_start(out=xps[0][:, 0:10, :], in_=src0[:, 0:10, :]).then_inc(s_xa, 16)
        nc.scalar.dma_start(out=wt[:, 3:, :], in_=w_ext[:, 3:, :]).then_inc(s_w2, 16)
        nc.gpsimd.dma_start(out=wt[:, 0:3, :], in_=w_ext[:, 0:3, :]).then_inc(s_w1, 16)
        nc.sync.dma_start(out=xps[0][:, 10:34, :], in_=src0[:, 10:34, :]).then_inc(s_xb, 16)
        nc.scalar.dma_start(out=xps[0][:, 34:HP, :], in_=src0[:, 34:HP, :]).then_inc(s_xc, 16)

        # ---- pre-block: the whole junk ramp bridge; the tensor's block-
        # entry-barrier arrival (~junk end) is the junk/real boundary, with
        # all first-chunk DMA sems already satisfied -> no PE gap.
        # banks[3] junk contents are cleared by chunk 3's start=True.
        for wi in range(N_JUNK_BIG):
            h = wi % 2
            c0 = h * C_IN
            nc.tensor.matmul(
                out=banks[3][h][:],
                lhsT=wt[c0 : c0 + C_IN, 0, :],
                rhs=xps[0][c0 : c0 + C_IN, 0:RCHUNK, 0:H],
                start=True,
                stop=True,
            )
        for wi in range(N_JUNK_SMALL):
            h = wi % 2
            c0 = h * C_IN
            nc.tensor.matmul(
                out=banks[3][h][:, 0:1, :],
                lhsT=wt[c0 : c0 + C_IN, 0, :],
                rhs=xps[0][c0 : c0 + C_IN, 0:1, 0:H],
                start=True,
                stop=True,
            )

        with nc.Block() as block:

            @block.sync
            def _(sync: bass.BassEngine):
                for p in range(2):
                    blocks = OBLOCKS0 if p == 0 else OBLOCKS1 + [(48, 52), (52, 56)]
                    for bi, (blo, bhi) in enumerate(blocks):
                        if p == 0:
                            ticks = bhi // RCHUNK
                        elif bhi <= 48:
                            ticks = NCHUNK + bhi // RCHUNK
                        else:
                            ticks = 14 if bhi == 52 else 15
                        sync.wait_ge(s_cp, ticks)
                        sync.dma_start(
                            out=dsts[2 * p][:, blo:bhi, :], in_=obs[p][0][bi][:]
                        ).then_inc(s_out, 16)
                sync.wait_ge(s_out, 16 * n_out_dmas)

            @block.scalar
            def _(scalar: bass.BassEngine):
                cp2 = 0
                for p in range(2):
                    blocks = OBLOCKS0 if p == 0 else OBLOCKS1
                    for ci in range(NCHUNK):
                        c = p * NCHUNK + ci
                        h0 = ci * RCHUNK
                        if p == 1 and ci == NCHUNK - 1:
                            break  # minichunks below
                        bi, blo, bhi = _block_of(blocks, h0)
                        scalar.wait_ge(s_mm, 2 * (c + 1))
                        scalar.copy(
                            out=obs[p][1][bi][:, h0 - blo : h0 - blo + RCHUNK, :],
                            in_=banks[c % 4][1][:],
                        ).then_inc(s_cp2, 1)
                        cp2 += 1
                        if h0 + RCHUNK == bhi:
                            scalar.wait_ge(s_cp2, cp2)
                            scalar.dma_start(
                                out=dsts[2 * p + 1][:, blo:bhi, :], in_=obs[p][1][bi][:]
                            ).then_inc(s_out, 16)
                # minichunk h1 copies; mini0's DMA goes to gpsimd so the
                # scalar chain after the last matmul is copy+DMA once
                for mc in range(2):
                    bi = len(OBLOCKS1) + mc
                    scalar.wait_ge(s_mm, 28 + 2 * mc)
                    scalar.copy(out=obs[1][1][bi][:], in_=minib[mc][1]).then_inc(s_cp2, 1)
                    cp2 += 1
                scalar.wait_ge(s_cp2, cp2)
                scalar.dma_start(
                    out=dsts[3][:, 52:56, :], in_=obs[1][1][len(OBLOCKS1) + 1][:]
                ).then_inc(s_out, 16)

            @block.gpsimd
            def _(gpsimd: bass.BassEngine):
                gpsimd.wait_ge(s_mm, 2)
                gpsimd.dma_start(out=xps[1][:, 0:12, :], in_=src1[:, 0:12, :]).then_inc(s_x1, 16)
                gpsimd.dma_start(out=xps[1][:, 12:34, :], in_=src1[:, 12:34, :]).then_inc(s_x1, 16)
                gpsimd.dma_start(out=xps[1][:, 34:HP, :], in_=src1[:, 34:HP, :]).then_inc(s_x1, 16)
                # mini0 h1 output DMA (scalar handles mini1's)
                gpsimd.wait_ge(s_cp2, 14)
                gpsimd.dma_start(
                    out=dsts[3][:, 48:52, :], in_=obs[1][1][len(OBLOCKS1)][:]
                ).then_inc(s_out, 16)

            @block.tensor
            def _(tensor: bass.BassEngine):
                for p in range(2):
                    for ci in range(NCHUNK):
                        c = p * NCHUNK + ci
                        h0 = ci * RCHUNK
                        if p == 1 and ci == NCHUNK - 1:
                            break  # minichunks below
                        if p == 0:
                            if ci == 0:
                                tensor.wait_ge(s_w1, 16)  # taps 0-2
                                tensor.wait_ge(s_xa, 16)  # rows [0,10)
                            elif ci == 1:
                                tensor.wait_ge(s_xb, 16)  # rows [10,34)
                            elif ci == 4:
                                tensor.wait_ge(s_xc, 16)  # rows [34,58)
                        else:
                            if ci == 0:
                                tensor.wait_ge(s_x1, 16)
                            elif ci == 1:
                                tensor.wait_ge(s_x1, 32)
                            elif ci == 4:
                                tensor.wait_ge(s_x1, 48)
                        if c >= 4:
                            # WAR: bank slot c%4 last used by chunk c-4
                            tensor.wait_ge(s_cp, c - 3)
                            tensor.wait_ge(s_cp2, c - 3)
                        for k in range(KS * KS):
                            di, dj = divmod(k, KS)
                            last = k == KS * KS - 1
                            if p == 0 and ci == 0 and k == 3:
                                tensor.wait_ge(s_w2, 16)  # taps 3-8
                            for half in range(2):
                                c0 = half * C_IN
                                mm = tensor.matmul(
                                    out=banks[c % 4][half][:],
                                    lhsT=wt[c0 : c0 + C_IN, k, :],
                                    rhs=xps[p][
                                        c0 : c0 + C_IN,
                                        h0 + di : h0 + di + RCHUNK,
                                        dj : dj + H,
                                    ],
                                    start=(k == 0),
                                    stop=last,
                                )
                                if last and half == 1:
                                    mm.then_inc(s_mm, 2)
                # pair-1 rows 48-56 as two 4-row minichunks
                for mc in range(2):
                    h0 = 48 + 4 * mc
                    if mc == 0:
                        # WAR: banks[3] last written by chunk 11 (tick 12),
                        # banks[1] by chunk 9 (tick 10) — both long done
                        tensor.wait_ge(s_cp, 12)
                        tensor.wait_ge(s_cp2, 12)
                    for k in range(KS * KS):
                        di, dj = divmod(k, KS)
                        last = k == KS * KS - 1
                        for half in range(2):
                            c0 = half * C_IN
                            mm = tensor.matmul(
                                out=minib[mc][half],
                                lhsT=wt[c0 : c0 + C_IN, k, :],
                                rhs=xps[1][
                                    c0 : c0 + C_IN,
                                    h0 + di : h0 + di + 4,
                                    dj : dj + H,
                                ],
                                start=(k == 0),
                                stop=last,
                            )
                            if last and half == 1:
                                mm.then_inc(s_mm, 2)

            @block.vector
            def _(vector: bass.BassEngine):
                for p in range(2):
                    blocks = OBLOCKS0 if p == 0 else OBLOCKS1
                    for ci in range(NCHUNK):
                        c = p * NCHUNK + ci
                        h0 = ci * RCHUNK
                        if p == 1 and ci == NCHUNK - 1:
                            break
                        bi, blo, bhi = _block_of(blocks, h0)
                        vector.wait_ge(s_mm, 2 * (c + 1))
                        vector.tensor_copy(
                            out=obs[p][0][bi][:, h0 - blo : h0 - blo + RCHUNK, :],
                            in_=banks[c % 4][0][:],
                        ).then_inc(s_cp, 1)
                for mc in range(2):
                    bi = len(OBLOCKS1) + mc
                    vector.wait_ge(s_mm, 28 + 2 * mc)
                    vector.tensor_copy(out=obs[1][0][bi][:], in_=minib[mc][0]).then_inc(s_cp, 1)

    return nc


def _prep_inputs(x, K, mm_dt=MM_DT):
    np_dt = mybir.dt.np(mm_dt)
    x = np.ascontiguousarray(np.asarray(x, dtype=np.float32))
    K = np.ascontiguousarray(np.asarray(K, dtype=np.float32))
    xpad = np.pad(x, ((0, 0), (0, 0), (1, 1), (1, 1))).astype(np_dt)
    Wt = K.reshape(KS * KS * C_IN, C_OUT).reshape(C_IN, KS * KS, C_OUT)
    Wrep = np.ascontiguousarray(np.concatenate([Wt, Wt], axis=0)).astype(np_dt)
    shards = xpad.reshape(N_CORES, BPC, C_IN, HP, HP)
    return [{"x": np.ascontiguousarray(shards[i]), "w": Wrep} for i in range(N_CORES)]


def run(x, K, trace=False, mm_dt=MM_DT):
    nc = build_nc(mm_dt)
    in_maps = _prep_inputs(x, K, mm_dt)
    res = run_bass_kernel_spmd(nc, in_maps, list(range(N_CORES)), trace=trace)
    out = np.concatenate([res.results[i]["out"] for i in range(N_CORES)], axis=0)
    return out.astype(np.float32), res


def kernel(x, K):
    out, _ = run(x, K, trace=False)
    return out


# revision 23
# speedup vs baseline: 1.1299x; 1.0117x over previous
"""Trainium2 Bass kernel for nn_Conv2d_24833500905755 (3x3 conv, B=32,
C_in=64, C_out=128, 56x56, pad 1, with the reference's mismatched
weight-flatten order).

Math: out[b,co,h,w] = sum_{c,di,dj} xpad[b,c,h+di,w+dj] * Wt[c,di*3+dj,co]
with Wt = K.reshape(576, C_OUT).reshape(C_IN, 9, C_OUT).

Data-parallel: 4 images per NeuronCore, 2 images packed on the
128-partition dim (fp16 matmuls, K=64 contraction per half, concurrent
PE row-group tiles). Raw-bass hand-scheduled engine programs.

v5 scheduling model (calibrated from perfetto traces):
  - All engine streams are released from the framework preamble at
    ~7.2-7.9us; the first PE instruction lands ~7.3us.  DMA chain per
    instruction: issue 0.6-0.7 + hwdge/dge 1.3 + transfer (360 GB/s
    aggregate) + 0.9 sem propagation -> first input pieces land ~10.5.
  - Critical pieces (xA rows 0:10 on sync, w taps 0-3 on scalar) are
    issued pre-block from the earliest slots; w taps 4-8 / xC second
    on the same engines; xB from gpsimd (released late ~7.9).
  - PE p-state: half clock for ~6us from first matmul, resets on idle.
    ALL junk runs pre-block so the tensor's block-entry-barrier arrival
    IS the junk/real boundary (~10.9), with DMA sems pre-satisfied ->
    no PE gap, ramp completes ~13.2.
  - Tail: pair-1 ends with two 4-row minichunks; copies split across
    vector (h0) / scalar (h1), DMAs across sync (h0) / gpsimd+scalar
    (h1) to minimize the exposed serial chain after the last matmul.

Output is fp16 on-chip and in HBM; host upcasts to fp32.
"""

from contextlib import ExitStack

import numpy as np

import concourse.bass as bass
import concourse.mybir as mybir
from concourse.bass_utils import run_bass_kernel_spmd

B, C_IN, C_OUT, H = 32, 64, 128, 56
KS = 3
N_CORES = 8
BPC = B // N_CORES
HP = H + 2
RCHUNK = 8
NCHUNK = H // RCHUNK          # 7 chunks/image
OBLOCKS0 = [(0, 24), (24, 40), (40, 48), (48, 56)]
OBLOCKS1 = [(0, 24), (24, 32), (32, 40), (40, 48)]  # + minis (48,52),(52,56)
MM_DT = mybir.dt.float16
OUT_DT = mybir.dt.float16
N_JUNK_BIG = 16               # pre-block 448-col junks (8 pair-slots)
N_JUNK_SMALL = 12             # pre-block 56-col junks (6 pair-slots, granular)


def _block_of(blocks, h0):
    for bi, (blo, bhi) in enumerate(blocks):
        if blo <= h0 < bhi:
            return bi, blo, bhi
    raise AssertionError(h0)


def build_nc(mm_dt=MM_DT):
    f32 = mybir.dt.float32
    nc = bass.Bass()
    x_ext = nc.declare_dram_parameter("x", [BPC, C_IN, HP, HP], mm_dt, isOutput=False)
    w_ext = nc.declare_dram_parameter("w", [2 * C_IN, KS * KS, C_OUT], mm_dt, isOutput=False)
    out_ext = nc.declare_dram_parameter("out", [BPC, C_OUT, H, H], OUT_DT, isOutput=True)

    # out DMAs: pair0 4 blocks x2 halves + pair1 (4 blocks + 2 minis) x2
    n_out_dmas = len(OBLOCKS0) * 2 + (len(OBLOCKS1) + 2) * 2

    with ExitStack() as ctx:
        wt = ctx.enter_context(nc.sbuf_tensor("wt", [2 * C_IN, KS * KS, C_OUT], mm_dt))
        xps = [
            ctx.enter_context(nc.sbuf_tensor(f"xp{p}", [2 * C_IN, HP, HP], mm_dt))
            for p in range(2)
        ]
        obs = [
            [
                [
                    ctx.enter_context(
                        nc.sbuf_tensor(f"ob_{p}_{h}_{bi}", [C_OUT, bhi - blo, H], OUT_DT)
                    )
                    for bi, (blo, bhi) in enumerate(
                        OBLOCKS0 if p == 0 else OBLOCKS1 + [(48, 52), (52, 56)]
                    )
                ]
                for h in range(2)
            ]
            for p in range(2)
        ]
        banks = [
            [
                ctx.enter_context(
                    nc.psum_tensor(f"ps_{s}_{h}", [C_OUT, RCHUNK, H], f32)
                )
                for h in range(2)
            ]
            for s in range(4)
        ]
        # minichunk banks alias retired full banks: mini0 -> banks[3]
        # (last used by chunk 11), mini1 -> banks[1] (chunk 9)
        minib = [
            [banks[3][h][:, 0:4, :] for h in range(2)],
            [banks[1][h][:, 0:4, :] for h in range(2)],
        ]
        s_w1 = ctx.enter_context(nc.semaphore("s_w1"))
        s_w2 = ctx.enter_context(nc.semaphore("s_w2"))
        s_junk = ctx.enter_context(nc.semaphore("s_junk"))
        s_xa = ctx.enter_context(nc.semaphore("s_xa"))
        s_xb = ctx.enter_context(nc.semaphore("s_xb"))
        s_xc = ctx.enter_context(nc.semaphore("s_xc"))
        s_x1 = ctx.enter_context(nc.semaphore("s_x1"))
        s_mm = ctx.enter_context(nc.semaphore("s_mm"))
        s_cp = ctx.enter_context(nc.semaphore("s_cp"))
        s_cp2 = ctx.enter_context(nc.semaphore("s_cp2"))
        s_out = ctx.enter_context(nc.semaphore("s_out"))

        src0 = x_ext[0:2].rearrange("b c h w -> (b c) h w")
        src1 = x_ext[2:4].rearrange("b c h w -> (b c) h w")
        dsts = [
            out_ext[i : i + 1].rearrange("b c h w -> (b c) h w") for i in range(BPC)
        ]

        # ---- pre-block: critical DMA issues ----
        # HWDGE descriptor-gen (~0.63us/DMA) is a single shared device and
        # serializes across sync/scalar DMAs in issue order; transfers then
        # share the wire.  Gen order: xA, w1, w2, xB, xC — and xB/xC issues
        # are gated on the tensor's junk-progress semaphore so their
        # transfers start only after the critical pieces' wires are done.
        nc.sync.dma_start(out=xps[0][:, 0:10, :], in_=src0[:, 0:10, :]).then_inc(s_xa, 16)
        nc.scalar.dma_start(out=wt[:, 0:3, :], in_=w_ext[:, 0:3, :]).then_inc(s_w1, 16)
        nc.scalar.dma_start(out=wt[:, 3:, :], in_=w_ext[:, 3:, :]).then_inc(s_w2, 16)
        nc.sync.wait_ge(s_junk, 4)
        nc.sync.dma_start(out=xps[0][:, 10:34, :], in_=src0[:, 10:34, :]).then_inc(s_xb, 16)
        nc.scalar.wait_ge(s_junk, 7)
        nc.scalar.dma_start(out=xps[0][:, 34:HP, :], in_=src0[:, 34:HP, :]).then_inc(s_xc, 16)

        # ---- pre-block: the whole junk ramp bridge; the tensor's block-
        # entry-barrier arrival (~junk end) is the junk/real boundary, with
        # all first-chunk DMA sems already satisfied -> no PE gap.
        # banks[3] junk contents are cleared by chunk 3's start=True.
        for wi in range(N_JUNK_BIG):
            h = wi % 2
            c0 = h * C_IN
            mm = nc.tensor.matmul(
                out=banks[3][h][:],
                lhsT=wt[c0 : c0 + C_IN, 0, :],
                rhs=xps[0][c0 : c0 + C_IN, 0:RCHUNK, 0:H],
                start=True,
                stop=True,
            )
            if wi % 2 == 1:
                mm.then_inc(s_junk, 1)  # pair clock for DMA staggering
        for wi in range(N_JUNK_SMALL):
            h = wi % 2
            c0 = h * C_IN
            nc.tensor.matmul(
                out=banks[3][h][:, 0:1, :],
                lhsT=wt[c0 : c0 + C_IN, 0, :],
                rhs=xps[0][c0 : c0 + C_IN, 0:1, 0:H],
                start=True,
                stop=True,
            )

        with nc.Block() as block:

            @block.sync
            def _(sync: bass.BassEngine):
                for p in range(2):
                    blocks = OBLOCKS0 if p == 0 else OBLOCKS1 + [(48, 52), (52, 56)]
                    for bi, (blo, bhi) in enumerate(blocks):
                        if p == 0:
                            ticks = bhi // RCHUNK
                        elif bhi <= 48:
                            ticks = NCHUNK + bhi // RCHUNK
                        else:
                            ticks = 14 if bhi == 52 else 15
                        sync.wait_ge(s_cp, ticks)
                        sync.dma_start(
                            out=dsts[2 * p][:, blo:bhi, :], in_=obs[p][0][bi][:]
                        ).then_inc(s_out, 16)
                sync.wait_ge(s_out, 16 * n_out_dmas)

            @block.scalar
            def _(scalar: bass.BassEngine):
                cp2 = 0
                for p in range(2):
                    blocks = OBLOCKS0 if p == 0 else OBLOCKS1
                    for ci in range(NCHUNK):
                        c = p * NCHUNK + ci
                        h0 = ci * RCHUNK
                        if p == 1 and ci == NCHUNK - 1:
                            break  # minichunks below
                        bi, blo, bhi = _block_of(blocks, h0)
                        scalar.wait_ge(s_mm, 2 * (c + 1))
                        scalar.copy(
                            out=obs[p][1][bi][:, h0 - blo : h0 - blo + RCHUNK, :],
                            in_=banks[c % 4][1][:],
                        ).then_inc(s_cp2, 1)
                        cp2 += 1
                        if h0 + RCHUNK == bhi:
                            scalar.wait_ge(s_cp2, cp2)
                            scalar.dma_start(
                                out=dsts[2 * p + 1][:, blo:bhi, :], in_=obs[p][1][bi][:]
                            ).then_inc(s_out, 16)
                # minichunk h1 copies; mini0's DMA goes to gpsimd so the
                # scalar chain after the last matmul is copy+DMA once
                for mc in range(2):
                    bi = len(OBLOCKS1) + mc
                    scalar.wait_ge(s_mm, 28 + 2 * mc)
                    scalar.copy(out=obs[1][1][bi][:], in_=minib[mc][1]).then_inc(s_cp2, 1)
                    cp2 += 1
                scalar.wait_ge(s_cp2, cp2)
                scalar.dma_start(
                    out=dsts[3][:, 52:56, :], in_=obs[1][1][len(OBLOCKS1) + 1][:]
                ).then_inc(s_out, 16)

            @block.gpsimd
            def _(gpsimd: bass.BassEngine):
                gpsimd.wait_ge(s_mm, 2)
                gpsimd.dma_start(out=xps[1][:, 0:12, :], in_=src1[:, 0:12, :]).then_inc(s_x1, 16)
                gpsimd.dma_start(out=xps[1][:, 12:34, :], in_=src1[:, 12:34, :]).then_inc(s_x1, 16)
                gpsimd.dma_start(out=xps[1][:, 34:HP, :], in_=src1[:, 34:HP, :]).then_inc(s_x1, 16)
                # mini0 h1 output DMA (scalar handles mini1's)
                gpsimd.wait_ge(s_cp2, 14)
                gpsimd.dma_start(
                    out=dsts[3][:, 48:52, :], in_=obs[1][1][len(OBLOCKS1)][:]
                ).then_inc(s_out, 16)

            @block.tensor
            def _(tensor: bass.BassEngine):
                for p in range(2):
                    for ci in range(NCHUNK):
                        c = p * NCHUNK + ci
                        h0 = ci * RCHUNK
                        if p == 1 and ci == NCHUNK - 1:
                            break  # minichunks below
                        if p == 0:
                            if ci == 0:
                                tensor.wait_ge(s_w1, 16)  # taps 0-2
                                tensor.wait_ge(s_xa, 16)  # rows [0,10)
                            elif ci == 1:
                                tensor.wait_ge(s_xb, 16)  # rows [10,34)
                            elif ci == 4:
                                tensor.wait_ge(s_xc, 16)  # rows [34,58)
                        else:
                            if ci == 0:
                                tensor.wait_ge(s_x1, 16)
                            elif ci == 1:
                                tensor.wait_ge(s_x1, 32)
                            elif ci == 4:
                                tensor.wait_ge(s_x1, 48)
                        if c >= 4:
                            # WAR: bank slot c%4 last used by chunk c-4
                            tensor.wait_ge(s_cp, c - 3)
                            tensor.wait_ge(s_cp2, c - 3)
                        for k in range(KS * KS):
                            di, dj = divmod(k, KS)
                            last = k == KS * KS - 1
                            if p == 0 and ci == 0 and k == 3:
                                tensor.wait_ge(s_w2, 16)  # taps 3-8
                            for half in range(2):
                                c0 = half * C_IN
                                mm = tensor.matmul(
                                    out=banks[c % 4][half][:],
                                    lhsT=wt[c0 : c0 + C_IN, k, :],
                                    rhs=xps[p][
                                        c0 : c0 + C_IN,
                                        h0 + di : h0 + di + RCHUNK,
                                        dj : dj + H,
                                    ],
                                    start=(k == 0),
                                    stop=last,
                                )
                                if last and half == 1:
                                    mm.then_inc(s_mm, 2)
                # pair-1 rows 48-56 as two 4-row minichunks
                for mc in range(2):
                    h0 = 48 + 4 * mc
                    if mc == 0:
                        # WAR: banks[3] last written by chunk 11 (tick 12),
                        # banks[1] by chunk 9 (tick 10) — both long done
                        tensor.wait_ge(s_cp, 12)
                        tensor.wait_ge(s_cp2, 12)
                    for k in range(KS * KS):
                        di, dj = divmod(k, KS)
                        last = k == KS * KS - 1
                        for half in range(2):
                            c0 = half * C_IN
                            mm = tensor.matmul(
                                out=minib[mc][half],
                                lhsT=wt[c0 : c0 + C_IN, k, :],
                                rhs=xps[1][
                                    c0 : c0 + C_IN,
                                    h0 + di : h0 + di + 4,
                                    dj : dj + H,
                                ],
                                start=(k == 0),
                                stop=last,
                            )
                            if last and half == 1:
                                mm.then_inc(s_mm, 2)

            @block.vector
            def _(vector: bass.BassEngine):
                for p in range(2):
                    blocks = OBLOCKS0 if p == 0 else OBLOCKS1
                    for ci in range(NCHUNK):
                        c = p * NCHUNK + ci
                        h0 = ci * RCHUNK
                        if p == 1 and ci == NCHUNK - 1:
                            break
                        bi, blo, bhi = _block_of(blocks, h0)
                        vector.wait_ge(s_mm, 2 * (c + 1))
                        vector.tensor_copy(
                            out=obs[p][0][bi][:, h0 - blo : h0 - blo + RCHUNK, :],
                            in_=banks[c % 4][0][:],
                        ).then_inc(s_cp, 1)
                for mc in range(2):
                    bi = len(OBLOCKS1) + mc
                    vector.wait_ge(s_mm, 28 + 2 * mc)
                    vector.tensor_copy(out=obs[1][0][bi][:], in_=minib[mc][0]).then_inc(s_cp, 1)

    return nc


def _prep_inputs(x, K, mm_dt=MM_DT):
    np_dt = mybir.dt.np(mm_dt)
    x = np.ascontiguousarray(np.asarray(x, dtype=np.float32))
    K = np.ascontiguousarray(np.asarray(K, dtype=np.float32))
    xpad = np.pad(x, ((0, 0), (0, 0), (1, 1), (1, 1))).astype(np_dt)
    Wt = K.reshape(KS * KS * C_IN, C_OUT).reshape(C_IN, KS * KS, C_OUT)
    Wrep = np.ascontiguousarray(np.concatenate([Wt, Wt], axis=0)).astype(np_dt)
    shards = xpad.reshape(N_CORES, BPC, C_IN, HP, HP)
    return [{"x": np.ascontiguousarray(shards[i]), "w": Wrep} for i in range(N_CORES)]


def run(x, K, trace=False, mm_dt=MM_DT):
    nc = build_nc(mm_dt)
    in_maps = _prep_inputs(x, K, mm_dt)
    res = run_bass_kernel_spmd(nc, in_maps, list(range(N_CORES)), trace=trace)
    out = np.concatenate([res.results[i]["out"] for i in range(N_CORES)], axis=0)
    return out.astype(np.float32), res


def kernel(x, K):
    out, _ = run(x, K, trace=False)
    return out


# revision 25
# speedup vs baseline: 1.1738x; 1.0389x over previous
"""Trainium2 Bass kernel for nn_Conv2d_24833500905755 (3x3 conv, B=32,
C_in=64, C_out=128, 56x56, pad 1, with the reference's mismatched
weight-flatten order).

Math: out[b,co,h,w] = sum_{c,di,dj} xpad[b,c,h+di,w+dj] * Wt[c,di*3+dj,co]
with Wt = K.reshape(576, C_OUT).reshape(C_IN, 9, C_OUT).

Data-parallel: 4 images per NeuronCore, 2 images packed on the
128-partition dim (fp16 matmuls, K=64 contraction per half, concurrent
PE row-group tiles). Raw-bass hand-scheduled engine programs.

v5 scheduling model (calibrated from perfetto traces):
  - All engine streams are released from the framework preamble at
    ~7.2-7.9us; the first PE instruction lands ~7.3us.  DMA chain per
    instruction: issue 0.6-0.7 + hwdge/dge 1.3 + transfer (360 GB/s
    aggregate) + 0.9 sem propagation -> first input pieces land ~10.5.
  - Critical pieces (xA rows 0:10 on sync, w taps 0-3 on scalar) are
    issued pre-block from the earliest slots; w taps 4-8 / xC second
    on the same engines; xB from gpsimd (released late ~7.9).
  - PE p-state: half clock for ~6us from first matmul, resets on idle.
    ALL junk runs pre-block so the tensor's block-entry-barrier arrival
    IS the junk/real boundary (~10.9), with DMA sems pre-satisfied ->
    no PE gap, ramp completes ~13.2.
  - Tail: pair-1 ends with two 4-row minichunks; copies split across
    vector (h0) / scalar (h1), DMAs across sync (h0) / gpsimd+scalar
    (h1) to minimize the exposed serial chain after the last matmul.

Output is fp16 on-chip and in HBM; host upcasts to fp32.
"""

from contextlib import ExitStack

import numpy as np

import concourse.bass as bass
import concourse.mybir as mybir
from concourse.bass_utils import run_bass_kernel_spmd

B, C_IN, C_OUT, H = 32, 64, 128, 56
KS = 3
N_CORES = 8
BPC = B // N_CORES
HP = H + 2
RCHUNK = 8
NCHUNK = H // RCHUNK          # 7 chunks/image
OBLOCKS0 = [(0, 24), (24, 40), (40, 48), (48, 56)]
OBLOCKS1 = [(0, 24), (24, 32), (32, 40), (40, 48)]  # + minis (48,52),(52,56)
MM_DT = mybir.dt.float16
OUT_DT = mybir.dt.float16
N_JUNK_BIG = 16               # pre-block 448-col junks (8 pair-slots)
N_JUNK_SMALL = 12             # pre-block 56-col junks (6 pair-slots, granular)


def _block_of(blocks, h0):
    for bi, (blo, bhi) in enumerate(blocks):
        if blo <= h0 < bhi:
            return bi, blo, bhi
    raise AssertionError(h0)


class _NoBlock:
    """Blockless emission: every cross-engine dependency is semaphore-
    managed, so the Block entry/exit all-engine barriers (~0.3-0.5us entry,
    ~0.4us exit on the critical tail) are pure overhead.  This shim keeps
    the @block.<engine> program structure but emits straight into each
    engine's main stream."""

    def __init__(self, nc):
        self.nc = nc

    def __enter__(self):
        return self

    def __exit__(self, *a):
        return False

    def sync(self, f):
        f(self.nc.sync)

    def scalar(self, f):
        f(self.nc.scalar)

    def gpsimd(self, f):
        f(self.nc.gpsimd)

    def tensor(self, f):
        f(self.nc.tensor)

    def vector(self, f):
        f(self.nc.vector)


def build_nc(mm_dt=MM_DT):
    f32 = mybir.dt.float32
    nc = bass.Bass()
    x_ext = nc.declare_dram_parameter("x", [BPC, C_IN, HP, HP], mm_dt, isOutput=False)
    w_ext = nc.declare_dram_parameter("w", [2 * C_IN, KS * KS, C_OUT], mm_dt, isOutput=False)
    out_ext = nc.declare_dram_parameter("out", [BPC, C_OUT, H, H], OUT_DT, isOutput=True)

    # out DMAs: pair0 4 blocks x2 halves + pair1 (4 blocks + 2 minis) x2
    n_out_dmas = len(OBLOCKS0) * 2 + (len(OBLOCKS1) + 2) * 2

    with ExitStack() as ctx:
        wt = ctx.enter_context(nc.sbuf_tensor("wt", [2 * C_IN, KS * KS, C_OUT], mm_dt))
        xps = [
            ctx.enter_context(nc.sbuf_tensor(f"xp{p}", [2 * C_IN, HP, HP], mm_dt))
            for p in range(2)
        ]
        obs = [
            [
                [
                    ctx.enter_context(
                        nc.sbuf_tensor(f"ob_{p}_{h}_{bi}", [C_OUT, bhi - blo, H], OUT_DT)
                    )
                    for bi, (blo, bhi) in enumerate(
                        OBLOCKS0 if p == 0 else OBLOCKS1 + [(48, 52), (52, 56)]
                    )
                ]
                for h in range(2)
            ]
            for p in range(2)
        ]
        banks = [
            [
                ctx.enter_context(
                    nc.psum_tensor(f"ps_{s}_{h}", [C_OUT, RCHUNK, H], f32)
                )
                for h in range(2)
            ]
            for s in range(4)
        ]
        # minichunk banks alias retired full banks: mini0 -> banks[3]
        # (last used by chunk 11), mini1 -> banks[1] (chunk 9)
        minib = [
            [banks[3][h][:, 0:4, :] for h in range(2)],
            [banks[1][h][:, 0:4, :] for h in range(2)],
        ]
        s_w1 = ctx.enter_context(nc.semaphore("s_w1"))
        s_w2 = ctx.enter_context(nc.semaphore("s_w2"))
        s_junk = ctx.enter_context(nc.semaphore("s_junk"))
        s_xa = ctx.enter_context(nc.semaphore("s_xa"))
        s_xb = ctx.enter_context(nc.semaphore("s_xb"))
        s_xc = ctx.enter_context(nc.semaphore("s_xc"))
        s_x1 = ctx.enter_context(nc.semaphore("s_x1"))
        s_mm = ctx.enter_context(nc.semaphore("s_mm"))
        s_cp = ctx.enter_context(nc.semaphore("s_cp"))
        s_cp2 = ctx.enter_context(nc.semaphore("s_cp2"))
        s_out = ctx.enter_context(nc.semaphore("s_out"))

        src0 = x_ext[0:2].rearrange("b c h w -> (b c) h w")
        src1 = x_ext[2:4].rearrange("b c h w -> (b c) h w")
        dsts = [
            out_ext[i : i + 1].rearrange("b c h w -> (b c) h w") for i in range(BPC)
        ]

        # ---- pre-block: critical DMA issues ----
        # HWDGE descriptor-gen (~0.63us/DMA) is a single shared device and
        # serializes across sync/scalar DMAs in issue order; transfers then
        # share the wire.  Gen order: xA, w1, w2, xB, xC — and xB/xC issues
        # are gated on the tensor's junk-progress semaphore so their
        # transfers start only after the critical pieces' wires are done.
        nc.sync.dma_start(out=xps[0][:, 0:10, :], in_=src0[:, 0:10, :]).then_inc(s_xa, 16)
        nc.scalar.dma_start(out=wt[:, 0:3, :], in_=w_ext[:, 0:3, :]).then_inc(s_w1, 16)
        nc.scalar.dma_start(out=wt[:, 3:, :], in_=w_ext[:, 3:, :]).then_inc(s_w2, 16)
        nc.sync.wait_ge(s_junk, 4)
        nc.sync.dma_start(out=xps[0][:, 10:34, :], in_=src0[:, 10:34, :]).then_inc(s_xb, 16)
        nc.scalar.wait_ge(s_junk, 7)
        nc.scalar.dma_start(out=xps[0][:, 34:HP, :], in_=src0[:, 34:HP, :]).then_inc(s_xc, 16)

        # ---- pre-block: the whole junk ramp bridge; the tensor's block-
        # entry-barrier arrival (~junk end) is the junk/real boundary, with
        # all first-chunk DMA sems already satisfied -> no PE gap.
        # banks[3] junk contents are cleared by chunk 3's start=True.
        for wi in range(N_JUNK_BIG):
            h = wi % 2
            c0 = h * C_IN
            mm = nc.tensor.matmul(
                out=banks[3][h][:],
                lhsT=wt[c0 : c0 + C_IN, 0, :],
                rhs=xps[0][c0 : c0 + C_IN, 0:RCHUNK, 0:H],
                start=True,
                stop=True,
            )
            if wi % 2 == 1:
                mm.then_inc(s_junk, 1)  # pair clock for DMA staggering
        for wi in range(N_JUNK_SMALL):
            h = wi % 2
            c0 = h * C_IN
            nc.tensor.matmul(
                out=banks[3][h][:, 0:1, :],
                lhsT=wt[c0 : c0 + C_IN, 0, :],
                rhs=xps[0][c0 : c0 + C_IN, 0:1, 0:H],
                start=True,
                stop=True,
            )

        with _NoBlock(nc) as block:

            @block.sync
            def _(sync: bass.BassEngine):
                for p in range(2):
                    blocks = OBLOCKS0 if p == 0 else OBLOCKS1 + [(48, 52), (52, 56)]
                    for bi, (blo, bhi) in enumerate(blocks):
                        if p == 0:
                            ticks = bhi // RCHUNK
                        elif bhi <= 48:
                            ticks = NCHUNK + bhi // RCHUNK
                        else:
                            ticks = 14 if bhi == 52 else 15
                        sync.wait_ge(s_cp, ticks)
                        sync.dma_start(
                            out=dsts[2 * p][:, blo:bhi, :], in_=obs[p][0][bi][:]
                        ).then_inc(s_out, 16)
                sync.wait_ge(s_out, 16 * n_out_dmas)

            @block.scalar
            def _(scalar: bass.BassEngine):
                cp2 = 0
                for p in range(2):
                    blocks = OBLOCKS0 if p == 0 else OBLOCKS1
                    for ci in range(NCHUNK):
                        c = p * NCHUNK + ci
                        h0 = ci * RCHUNK
                        if p == 1 and ci == NCHUNK - 1:
                            break  # minichunks below
                        bi, blo, bhi = _block_of(blocks, h0)
                        scalar.wait_ge(s_mm, 2 * (c + 1))
                        scalar.copy(
                            out=obs[p][1][bi][:, h0 - blo : h0 - blo + RCHUNK, :],
                            in_=banks[c % 4][1][:],
                        ).then_inc(s_cp2, 1)
                        cp2 += 1
                        if h0 + RCHUNK == bhi:
                            scalar.wait_ge(s_cp2, cp2)
                            scalar.dma_start(
                                out=dsts[2 * p + 1][:, blo:bhi, :], in_=obs[p][1][bi][:]
                            ).then_inc(s_out, 16)
                # minichunk h1 copies; mini0's DMA goes to gpsimd so the
                # scalar chain after the last matmul is copy+DMA once
                for mc in range(2):
                    bi = len(OBLOCKS1) + mc
                    scalar.wait_ge(s_mm, 28 + 2 * mc)
                    scalar.copy(out=obs[1][1][bi][:], in_=minib[mc][1]).then_inc(s_cp2, 1)
                    cp2 += 1
                scalar.wait_ge(s_cp2, cp2)
                scalar.dma_start(
                    out=dsts[3][:, 52:56, :], in_=obs[1][1][len(OBLOCKS1) + 1][:]
                ).then_inc(s_out, 16)

            @block.gpsimd
            def _(gpsimd: bass.BassEngine):
                gpsimd.wait_ge(s_mm, 2)
                gpsimd.dma_start(out=xps[1][:, 0:12, :], in_=src1[:, 0:12, :]).then_inc(s_x1, 16)
                gpsimd.dma_start(out=xps[1][:, 12:34, :], in_=src1[:, 12:34, :]).then_inc(s_x1, 16)
                gpsimd.dma_start(out=xps[1][:, 34:HP, :], in_=src1[:, 34:HP, :]).then_inc(s_x1, 16)
                # mini0 h1 output DMA (scalar handles mini1's)
                gpsimd.wait_ge(s_cp2, 14)
                gpsimd.dma_start(
                    out=dsts[3][:, 48:52, :], in_=obs[1][1][len(OBLOCKS1)][:]
                ).then_inc(s_out, 16)

            @block.tensor
            def _(tensor: bass.BassEngine):
                for p in range(2):
                    for ci in range(NCHUNK):
                        c = p * NCHUNK + ci
                        h0 = ci * RCHUNK
                        if p == 1 and ci == NCHUNK - 1:
                            break  # minichunks below
                        if p == 0:
                            if ci == 0:
                                tensor.wait_ge(s_w1, 16)  # taps 0-2
                                tensor.wait_ge(s_xa, 16)  # rows [0,10)
                            elif ci == 1:
                                tensor.wait_ge(s_xb, 16)  # rows [10,34)
                            elif ci == 4:
                                tensor.wait_ge(s_xc, 16)  # rows [34,58)
                        else:
                            if ci == 0:
                                tensor.wait_ge(s_x1, 16)
                            elif ci == 1:
                                tensor.wait_ge(s_x1, 32)
                            elif ci == 4:
                                tensor.wait_ge(s_x1, 48)
                        if c >= 4:
                            # WAR: bank slot c%4 last used by chunk c-4
                            tensor.wait_ge(s_cp, c - 3)
                            tensor.wait_ge(s_cp2, c - 3)
                        for k in range(KS * KS):
                            di, dj = divmod(k, KS)
                            last = k == KS * KS - 1
                            if p == 0 and ci == 0 and k == 3:
                                tensor.wait_ge(s_w2, 16)  # taps 3-8
                            for half in range(2):
                                c0 = half * C_IN
                                mm = tensor.matmul(
                                    out=banks[c % 4][half][:],
                                    lhsT=wt[c0 : c0 + C_IN, k, :],
                                    rhs=xps[p][
                                        c0 : c0 + C_IN,
                                        h0 + di : h0 + di + RCHUNK,
                                        dj : dj + H,
                                    ],
                                    start=(k == 0),
                                    stop=last,
                                )
                                if last and half == 1:
                                    mm.then_inc(s_mm, 2)
                # pair-1 rows 48-56 as two 4-row minichunks
                for mc in range(2):
                    h0 = 48 + 4 * mc
                    if mc == 0:
                        # WAR: banks[3] last written by chunk 11 (tick 12),
                        # banks[1] by chunk 9 (tick 10) — both long done
                        tensor.wait_ge(s_cp, 12)
                        tensor.wait_ge(s_cp2, 12)
                    for k in range(KS * KS):
                        di, dj = divmod(k, KS)
                        last = k == KS * KS - 1
                        for half in range(2):
                            c0 = half * C_IN
                            mm = tensor.matmul(
                                out=minib[mc][half],
                                lhsT=wt[c0 : c0 + C_IN, k, :],
                                rhs=xps[1][
                                    c0 : c0 + C_IN,
                                    h0 + di : h0 + di + 4,
                                    dj : dj + H,
                                ],
                                start=(k == 0),
                                stop=last,
                            )
                            if last and half == 1:
                                mm.then_inc(s_mm, 2)

            @block.vector
            def _(vector: bass.BassEngine):
                for p in range(2):
                    blocks = OBLOCKS0 if p == 0 else OBLOCKS1
                    for ci in range(NCHUNK):
                        c = p * NCHUNK + ci
                        h0 = ci * RCHUNK
                        if p == 1 and ci == NCHUNK - 1:
                            break
                        bi, blo, bhi = _block_of(blocks, h0)
                        vector.wait_ge(s_mm, 2 * (c + 1))
                        vector.tensor_copy(
                            out=obs[p][0][bi][:, h0 - blo : h0 - blo + RCHUNK, :],
                            in_=banks[c % 4][0][:],
                        ).then_inc(s_cp, 1)
                for mc in range(2):
                    bi = len(OBLOCKS1) + mc
                    vector.wait_ge(s_mm, 28 + 2 * mc)
                    vector.tensor_copy(out=obs[1][0][bi][:], in_=minib[mc][0]).then_inc(s_cp, 1)

    return nc


def _prep_inputs(x, K, mm_dt=MM_DT):
    np_dt = mybir.dt.np(mm_dt)
    x = np.ascontiguousarray(np.asarray(x, dtype=np.float32))
    K = np.ascontiguousarray(np.asarray(K, dtype=np.float32))
    xpad = np.pad(x, ((0, 0), (0, 0), (1, 1), (1, 1))).astype(np_dt)
    Wt = K.reshape(KS * KS * C_IN, C_OUT).reshape(C_IN, KS * KS, C_OUT)
    Wrep = np.ascontiguousarray(np.concatenate([Wt, Wt], axis=0)).astype(np_dt)
    shards = xpad.reshape(N_CORES, BPC, C_IN, HP, HP)
    return [{"x": np.ascontiguousarray(shards[i]), "w": Wrep} for i in range(N_CORES)]


def run(x, K, trace=False, mm_dt=MM_DT):
    nc = build_nc(mm_dt)
    in_maps = _prep_inputs(x, K, mm_dt)
    res = run_bass_kernel_spmd(nc, in_maps, list(range(N_CORES)), trace=trace)
    out = np.concatenate([res.results[i]["out"] for i in range(N_CORES)], axis=0)
    return out.astype(np.float32), res


def kernel(x, K):
    out, _ = run(x, K, trace=False)
    return out
